# revision 4
# baseline (speedup 1.0000x reference)
"""Multi-head attention (B=4, N=2048, D=1024, H=16) on 8 TRN2 NeuronCores.

Sharding: DP=4 over batch x TP=2 over heads (megatron style).
  core c = 2*batch + j   (j in {0,1} = head-group half)
  - inputs:  x[batch] transposed -> xT [D, N]  (bf16)
  - W_qkv column-sharded: wq/wk/wv = W_qkv[:, {q,k,v} block, heads j*8:(j+1)*8]
  - W_proj row-sharded:   wp = W_proj[j*512:(j+1)*512, :]
  - per-core partial out [N, D]; host sums the TP pair (+ bias via per-core
    bias input that is b_proj on j==0 and zeros on j==1).

Per-core pipeline (bf16 matmuls, fp32 PSUM):
  - qT/kT [feat, tok] feature-major, head pairs stacked on partitions
    (head e in {0,1} of a pair occupies partitions e*64:(e+1)*64).
  - S^T chunk [128 keys, 512 toks] = kT_chunk.T @ qT  (K=64). The two heads
    of a pair are emitted instruction-interleaved with EXPLICIT
    tile_position (0,0)/(64,0) so they row-tile-pack into the PE array
    concurrently.
  - exp on ScalarE in [128, 2*512] mega-chunks (2 key chunks per activation
    instruction to amortize the per-instruction access latency).
  - A@V feature-major: stationary = V augmented with a ones column
    [128 keys, 65] (cheap 65-cycle weight load), moving = exp(S^T)
    [128 keys, 512 toks] -> accumulates out [64 feat | den, 512 toks] in
    PSUM; softmax denominators ride along as output row 64. The previous
    combo's A@V is spread 6 matmuls per S-mega so the PE never emits long
    bursts that starve ScalarE or stall on PSUM recycling.
  - normalize: den rows DMA'd straight from PSUM to DRAM, then a stride-0
    partition-broadcast read builds a [128, 512] reciprocal source (both
    heads at once -> single DVE reciprocal), multiply writes the proj lhsT.
  - proj: out[tok block] = attT_chunk.T @ wp + bias; both 512-wide halves
    emitted adjacently per stationary chunk (weight-load reuse) into two
    alternating PSUM banks.
  - 8 junk warmup matmuls open every repetition so the PE HAM clock-gate
    stays warm across the input-DMA bubble.
"""

import numpy as np

B, N, D, H = 4, 2048, 1024, 16
HD = 64
NCORES = 8
TP = 2
HLOC = H // TP          # 8 heads per core
FDIM = HLOC * HD        # 512

_PROG_CACHE = {}


def _build_program(tok, d, h_loc, hd, debug=False, repeat=1):
    """Build the single-core Bass/Tile program (same program runs SPMD on all cores)."""
    import concourse.tile as tile
    from concourse import bacc, mybir

    f32 = mybir.dt.float32
    bf16 = mybir.dt.bfloat16
    Exp = mybir.ActivationFunctionType.Exp

    P = 128
    DC = d // P                 # qkv contraction chunks (8)
    NP = h_loc // 2             # head pairs (4)
    KC = tok // P               # key chunks (16)
    NMEG = KC // 2              # S/exp mega chunks (8), 2 key chunks each
    fdim = h_loc * hd           # local feature dim (512)
    FC = fdim // P              # proj contraction chunks (4)
    QT = 512                    # query tile
    QH = tok // QT              # query tiles per core (4)
    TC = QT // P                # token blocks per query tile (4)
    scale = float(hd) ** -0.5

    nc = bacc.Bacc("TRN2", target_bir_lowering=False, debug=debug)

    xT = nc.dram_tensor("xT", [d, tok], bf16, kind="ExternalInput")
    wq = nc.dram_tensor("wq", [d, fdim], bf16, kind="ExternalInput")
    wk = nc.dram_tensor("wk", [d, fdim], bf16, kind="ExternalInput")
    wv = nc.dram_tensor("wv", [d, fdim], bf16, kind="ExternalInput")
    wp = nc.dram_tensor("wp", [fdim, d], bf16, kind="ExternalInput")
    bias = nc.dram_tensor("bias", [P, d], f32, kind="ExternalInput")
    # bf16 partials: host upcasts + sums the TP pair in f32
    out = nc.dram_tensor("out", [tok, d], bf16, kind="ExternalOutput")

    with tile.TileContext(nc) as tc:
        with (
            tc.tile_pool(name="sing", bufs=1) as sing,
            tc.tile_pool(name="psS", bufs=2, space="PSUM") as psS,
            tc.tile_pool(name="psO", bufs=2, space="PSUM") as psO,
            tc.tile_pool(name="psW", bufs=2, space="PSUM") as psW,
            tc.tile_pool(name="atp", bufs=20) as atp,
            tc.tile_pool(name="work", bufs=4) as work,
            tc.tile_pool(name="outp", bufs=4) as outp,
            tc.tile_pool(name="dscr", bufs=8, space="DRAM") as dscr,
        ):
          # HAM warmup fodder: constant SBUF tile, written once, read by the
          # junk matmuls that open every repetition.
          junk = sing.tile([P, 512], bf16)
          nc.vector.memset(junk, 0.25)
          for _rep in range(repeat):
            # junk matmuls keep the PE busy (HAM stays at K=8/8) while the
            # per-rep input DMAs stream in; they have no DMA dependencies.
            for _w in range(8):
                wps = psW.tile([P, 512], f32, tag="ps", name="wps")
                nc.tensor.matmul(wps, junk[:, 0:128], junk,
                                 start=True, stop=True)

            # ---- resident loads, first-needed first ------------------------
            wk_sb = sing.tile([P, DC, fdim], bf16)
            wq_sb = sing.tile([P, DC, fdim], bf16)
            wv_sb = sing.tile([P, DC, fdim], bf16)
            xT_sb = sing.tile([P, DC, tok], bf16)
            dma_engs = (nc.sync, nc.gpsimd, nc.scalar)

            def wload(w_dram, w_sb, pr, eng):
                eng.dma_start(
                    out=w_sb[:, :, pr * P:(pr + 1) * P],
                    in_=w_dram[:, pr * P:(pr + 1) * P].rearrange(
                        "(c p) m -> p c m", p=P))

            def xload(ts, qn):
                t0 = ts * (tok // 4)
                t1 = t0 + tok // 4
                for c in range(DC):
                    eng = dma_engs[(qn + c) % 3]
                    eng.dma_start(
                        out=xT_sb[:, c, t0:t1],
                        in_=xT[c * P:(c + 1) * P, t0:t1])

            wload(wk, wk_sb, 0, nc.gpsimd)
            wload(wq, wq_sb, 0, nc.sync)
            nc.scalar.dma_start(
                out=wv_sb, in_=wv[:, :].rearrange("(c p) m -> p c m", p=P))
            xload(0, 0)
            xload(1, 1)
            for pr in range(1, NP):
                wload(wk, wk_sb, pr, nc.gpsimd)
                wload(wq, wq_sb, pr, nc.sync)
            xload(2, 2)
            xload(3, 0)
            wp_sb = sing.tile([P, FC, d], bf16)
            nc.sync.dma_start(out=wp_sb, in_=wp[:, :].rearrange("(c p) m -> p c m", p=P))
            bias_sb = sing.tile([P, d], f32)
            nc.gpsimd.dma_start(out=bias_sb, in_=bias[:, :])

            qT_sb = sing.tile([P, NP, tok], bf16)
            kT_sb = sing.tile([P, NP, tok], bf16)
            vaug_sb = sing.tile([P, KC, NP, 2, hd + 1], bf16)
            # only the ones column needs init; v copies fill [0:hd]
            nc.vector.memset(vaug_sb[:, :, :, :, hd], 1.0)
            attT_sb = sing.tile([P, 2, NP, QT], bf16)

            # warm the exp activation table while DMAs stream (no DMA dep)
            warm_in = work.tile([P, 1], f32, tag="warm_in")
            nc.vector.memset(warm_in, 0.0)
            warm = work.tile([P, 1], f32, tag="warm")
            nc.scalar.activation(warm, warm_in, Exp)

            def qk_pair(jobs):
                """jobs: list of (w_sb, dst, p, n0) 512-token chunks. Emits
                the dc contraction chains interleaved across two PSUM banks
                (back-to-back matmuls into one bank stall the PE). A single
                job is split into two half-token chunks instead."""
                if len(jobs) == 1:
                    w_sb, dst, p, n0 = jobs[0]
                    psA = psW.tile([P, 512], f32, tag="ps", name="psA")
                    psB = psW.tile([P, 512], f32, tag="ps", name="psB")
                    for c in range(DC):
                        for ps, o in ((psA, 0), (psB, 256)):
                            nc.tensor.matmul(
                                ps[:, 0:256],
                                w_sb[:, c, p * P:(p + 1) * P],
                                xT_sb[:, c, n0 + o:n0 + o + 256],
                                start=(c == 0),
                                stop=(c == DC - 1),
                            )
                    nc.vector.tensor_copy(dst[:, p, n0:n0 + 256], psA[:, 0:256])
                    nc.vector.tensor_copy(dst[:, p, n0 + 256:n0 + 512],
                                          psB[:, 0:256])
                    return
                assert len(jobs) == 2
                tiles = [psW.tile([P, 512], f32, tag="ps", name=f"psj{i}")
                         for i in range(2)]
                for c in range(DC):
                    for (w_sb, dst, p, n0), ps in zip(jobs, tiles):
                        nc.tensor.matmul(
                            ps,
                            w_sb[:, c, p * P:(p + 1) * P],
                            xT_sb[:, c, n0:n0 + 512],
                            start=(c == 0),
                            stop=(c == DC - 1),
                        )
                for (w_sb, dst, p, n0), ps in zip(jobs, tiles):
                    nc.vector.tensor_copy(dst[:, p, n0:n0 + 512], ps)

            def v_pair(tb_a, tb_b):
                """vaug[:, tb, :, :, 0:hd] for two token blocks, all pairs at
                once (N=512 streams), chains interleaved across two banks."""
                tiles = [psW.tile([P, 512], f32, tag="ps", name=f"psv{i}")
                         for i in range(2)]
                for c in range(DC):
                    for tb, ps in zip((tb_a, tb_b), tiles):
                        nc.tensor.matmul(
                            ps,
                            xT_sb[:, c, tb * P:(tb + 1) * P],
                            wv_sb[:, c, :],
                            start=(c == 0),
                            stop=(c == DC - 1),
                        )
                for tb, ps in zip((tb_a, tb_b), tiles):
                    nc.vector.tensor_copy(
                        vaug_sb[:, tb, :, :, 0:hd],
                        ps.rearrange("p (pr e f) -> p pr e f", pr=NP, e=2),
                    )

            def proj_tb(qh_, tb):
                """out[tb block] = attT.T @ wp + bias. Both 512-wide halves
                are emitted adjacently per stationary chunk (weight reuse),
                chains interleaved across two PSUM banks."""
                r0 = qh_ * QT + tb * P
                pps = [psW.tile([P, 512], f32, tag="ps", name=f"pp{i}")
                       for i in range(2)]
                for fc in range(FC):
                    for i, pp in enumerate(pps):
                        nc.tensor.matmul(
                            pp,
                            attT_sb[:, qh_ % 2, fc, tb * P:(tb + 1) * P],
                            wp_sb[:, fc, i * 512:(i + 1) * 512],
                            start=(fc == 0),
                            stop=(fc == FC - 1),
                        )
                for i, pp in enumerate(pps):
                    ot = outp.tile([P, 512], bf16, tag="ot")
                    nc.vector.tensor_add(ot, pp, bias_sb[:, i * 512:(i + 1) * 512])
                    oeng = nc.sync if (tb + i) % 2 == 0 else nc.gpsimd
                    oeng.dma_start(out=out[r0:r0 + P, i * 512:(i + 1) * 512],
                                   in_=ot)

            def s_mega_pair(p, q0, j):
                ss = [psS.tile([P, 2, QT], f32, tag="ss", name=f"ss{e}")
                      for e in range(2)]
                for jj in range(2):
                    kc = 2 * j + jj
                    # explicit tile_position + per-instruction head
                    # interleave: the e0/e1 matmuls land on different PE
                    # row-groups and overlap
                    for e in range(2):
                        nc.tensor.matmul(
                            ss[e][:, jj, :],
                            kT_sb[e * hd:(e + 1) * hd, p, kc * P:(kc + 1) * P],
                            qT_sb[e * hd:(e + 1) * hd, p, q0:q0 + QT],
                            start=True,
                            stop=True,
                            tile_position=(e * hd, 0),
                        )
                ats = []
                for e in range(2):
                    at = atp.tile([P, 2, QT], bf16, tag="at", name=f"at{e}")
                    nc.scalar.activation(at, ss[e], Exp, scale=scale)
                    ats.append(at)
                return tuple(ats)

            def av_chunk(po, p, at_list, e, kc0, n):
                """n steps of one head's A@V accumulation chain."""
                for kc in range(kc0, kc0 + n):
                    nc.tensor.matmul(
                        po[e][0:hd + 1, :],
                        vaug_sb[:, kc, p, e, :],
                        at_list[kc // 2][e][:, kc % 2, :],
                        start=(kc == 0),
                        stop=(kc == KC - 1),
                    )

            def tail(pend):
                # normalize: den rows -> DRAM -> stride-0 partition-broadcast
                # read -> ONE [128, 512] reciprocal -> in-place multiply.
                # The unnormalized po is copied into attT up front so the
                # PSUM banks free without waiting on the DMA round trip.
                qh_t, p_t, po = pend[0], pend[1], pend[2]
                sums_dr = dscr.tile([2, QT], f32, tag="sums_dr")
                att = [attT_sb[e * hd:(e + 1) * hd, qh_t % 2, p_t, :]
                       for e in range(2)]
                for e in range(2):
                    den_sb = work.tile([hd + 1, QT], f32, tag=f"den{e}")
                    nc.vector.tensor_copy(
                        den_sb[hd:hd + 1, :], po[e][hd:hd + 1, :])
                    nc.vector.tensor_copy(att[e], po[e][0:hd, :])
                    nc.gpsimd.dma_start(out=sums_dr[e:e + 1, :],
                                        in_=den_sb[hd:hd + 1, :])
                rec_src = work.tile([P, QT], f32, tag="rsrc")
                for e in range(2):
                    nc.gpsimd.dma_start(
                        out=rec_src[e * hd:(e + 1) * hd, :],
                        in_=sums_dr[e:e + 1, :].partition_broadcast(hd))
                rec = work.tile([P, QT], f32, tag="rec")
                nc.vector.reciprocal_approx_fast(rec, rec_src)
                for e in range(2):
                    nc.vector.tensor_mul(
                        att[e], att[e], rec[e * hd:(e + 1) * hd, :])

            # ---- main loop: software-pipelined combos ----------------------
            # Per combo, S megas stream so ScalarE never starves; the
            # previous combo's A@V rides along 6 matmuls per mega, its tail
            # lands at mega 5, and the next combo's q/k chunks plus the
            # (remapped) proj blocks fill the remaining PE slots.
            combos = [(qh, p) for qh in range(QH) for p in range(NP)]
            qk_pair([(wk_sb, kT_sb, 0, 0), (wq_sb, qT_sb, 0, 0)])
            for g in range(4):
                v_pair(2 * g, 2 * g + 1)
            prev = None   # (qh, p, at_list): combo whose A@V+tail is deferred
            for ci, (qh, p) in enumerate(combos):
                q0 = qh * QT
                at_list = []
                po_prev = None
                for j in range(NMEG):
                    at_list.append(s_mega_pair(p, q0, j))
                    if prev is not None:
                        if j == 0:
                            po_prev = [psO.tile([P, QT], f32, tag="po",
                                                name=f"po{e}")
                                       for e in range(2)]
                        if j <= 4:
                            for e in range(2):
                                av_chunk(po_prev, prev[1], prev[2], e,
                                         3 * j, 3)
                        elif j == 5:
                            for e in range(2):
                                av_chunk(po_prev, prev[1], prev[2], e, 15, 1)
                            tail((prev[0], prev[1], po_prev))
                    if j == 1 and qh == 0:
                        qk_pair([(wk_sb, kT_sb, p, 512)])
                    elif j == 2:
                        if qh == 0:
                            qk_pair([(wk_sb, kT_sb, p, 1024)])
                        if ci + 1 < len(combos):
                            qh_n, p_n = combos[ci + 1]
                            qk_pair([(wq_sb, qT_sb, p_n, qh_n * QT)])
                    elif j == 3:
                        if qh == 0:
                            jobs = [(wk_sb, kT_sb, p, 1536)]
                            if p + 1 < NP:
                                jobs.append((wk_sb, kT_sb, p + 1, 0))
                            qk_pair(jobs)
                        elif p >= 1:
                            proj_tb(qh - 1, p - 1)
                        elif qh >= 2:
                            proj_tb(qh - 2, 3)
                    elif j == 5 and qh == 0 and ci == 0:
                        v_pair(8, 9)
                        v_pair(10, 11)
                    elif j == 6 and qh == 0 and ci == 0:
                        v_pair(12, 13)
                        v_pair(14, 15)
                prev = (qh, p, at_list)
            # epilogue: final combo's A@V interleaved with the independent
            # proj block, tail, then the last query tile's proj
            po_prev = [psO.tile([P, QT], f32, tag="po", name=f"po{e}")
                       for e in range(2)]
            for e in range(2):
                av_chunk(po_prev, prev[1], prev[2], e, 0, 8)
            proj_tb(QH - 2, 3)
            for e in range(2):
                av_chunk(po_prev, prev[1], prev[2], e, 8, 8)
            tail((prev[0], prev[1], po_prev))
            for tb in range(TC):
                proj_tb(QH - 1, tb)

    nc.compile()
    return nc


def get_program(tok=N, d=D, h_loc=HLOC, hd=HD, debug=False, repeat=1):
    key = (tok, d, h_loc, hd, debug, repeat)
    if key not in _PROG_CACHE:
        _PROG_CACHE[key] = _build_program(tok, d, h_loc, hd, debug=debug,
                                          repeat=repeat)
    return _PROG_CACHE[key]


def make_in_maps(inputs_f32, W_qkv, W_proj, b_proj):
    """Shard full inputs into the 8 per-core input dicts."""
    import ml_dtypes

    bf16 = ml_dtypes.bfloat16
    in_maps = []
    for core in range(NCORES):
        b_idx, j = divmod(core, TP)
        f0, f1 = j * FDIM, (j + 1) * FDIM
        xT = np.ascontiguousarray(inputs_f32[b_idx].T).astype(bf16)
        wq_s = np.ascontiguousarray(W_qkv[:, f0:f1]).astype(bf16)
        wk_s = np.ascontiguousarray(W_qkv[:, D + f0:D + f1]).astype(bf16)
        wv_s = np.ascontiguousarray(W_qkv[:, 2 * D + f0:2 * D + f1]).astype(bf16)
        wp_s = np.ascontiguousarray(W_proj[f0:f1, :]).astype(bf16)
        if j == 0:
            bias_rep = np.broadcast_to(b_proj.astype(np.float32), (128, D)).copy()
        else:
            bias_rep = np.zeros((128, D), np.float32)
        in_maps.append(
            {"xT": xT, "wq": wq_s, "wk": wk_s, "wv": wv_s, "wp": wp_s,
             "bias": bias_rep}
        )
    return in_maps


def kernel(inputs, W_qkv, W_proj, b_proj):
    from concourse.bass_utils import run_bass_kernel_spmd

    inputs = np.asarray(inputs, dtype=np.float32)
    W_qkv = np.asarray(W_qkv, dtype=np.float32)
    W_proj = np.asarray(W_proj, dtype=np.float32)
    b_proj = np.asarray(b_proj, dtype=np.float32)

    nc = get_program()
    in_maps = make_in_maps(inputs, W_qkv, W_proj, b_proj)
    res = run_bass_kernel_spmd(nc, in_maps, core_ids=list(range(NCORES)))
    outs = [r["out"].astype(np.float32) for r in res.results]
    full = np.stack([outs[TP * b] + outs[TP * b + 1] for b in range(B)], axis=0)
    return full


# revision 7
# speedup vs baseline: 1.2856x; 1.2856x over previous
"""Multi-head attention (B=4, N=2048, D=1024, H=16) on 8 TRN2 NeuronCores.

Sharding: DP=4 over batch x TP=2 over heads (megatron style).
  core c = 2*batch + j   (j in {0,1} = head-group half)
  - inputs:  x[batch] transposed -> xT [D, N]  (bf16)
  - W_qkv column-sharded: wq/wk/wv = W_qkv[:, {q,k,v} block, heads j*8:(j+1)*8]
  - W_proj row-sharded:   wp = W_proj[j*512:(j+1)*512, :]
  - per-core partial out [N, D]; host sums the TP pair (+ bias via per-core
    bias input that is b_proj on j==0 and zeros on j==1).

Per-core pipeline (bf16 matmuls, fp32 PSUM):
  - qT/kT [feat, tok] feature-major, head pairs stacked on partitions
    (head e in {0,1} of a pair occupies partitions e*64:(e+1)*64).
  - S^T chunk [128 keys, 512 toks] = kT_chunk.T @ qT  (K=64). The two heads
    of a pair are emitted instruction-interleaved with EXPLICIT
    tile_position (0,0)/(64,0) so they row-tile-pack into the PE array
    concurrently.
  - exp on ScalarE in [128, 2*512] mega-chunks (2 key chunks per activation
    instruction to amortize the per-instruction access latency).
  - A@V feature-major: stationary = V augmented with a ones column
    [128 keys, 65] (cheap 65-cycle weight load), moving = exp(S^T)
    [128 keys, 512 toks] -> accumulates out [64 feat | den, 512 toks] in
    PSUM; softmax denominators ride along as output row 64. The previous
    combo's A@V is spread 6 matmuls per S-mega so the PE never emits long
    bursts that starve ScalarE or stall on PSUM recycling.
  - normalize: den rows DMA'd straight from PSUM to DRAM, then a stride-0
    partition-broadcast read builds a [128, 512] reciprocal source (both
    heads at once -> single DVE reciprocal), multiply writes the proj lhsT.
  - proj: out[tok block] = attT_chunk.T @ wp + bias; both 512-wide halves
    emitted adjacently per stationary chunk (weight-load reuse) into two
    alternating PSUM banks.
  - 8 junk warmup matmuls open every repetition so the PE HAM clock-gate
    stays warm across the input-DMA bubble.
"""

import numpy as np

B, N, D, H = 4, 2048, 1024, 16
HD = 64
NCORES = 8
TP = 2
HLOC = H // TP          # 8 heads per core
FDIM = HLOC * HD        # 512

_PROG_CACHE = {}


def _build_program(tok, d, h_loc, hd, debug=False, repeat=1):
    """Build the single-core Bass/Tile program (same program runs SPMD on all cores)."""
    import concourse.tile as tile
    from concourse import bacc, mybir

    f32 = mybir.dt.float32
    bf16 = mybir.dt.bfloat16
    Exp = mybir.ActivationFunctionType.Exp

    P = 128
    DC = d // P                 # qkv contraction chunks (8)
    NP = h_loc // 2             # head pairs (4)
    KC = tok // P               # key chunks (16)
    NMEG = KC // 2              # S/exp mega chunks (8), 2 key chunks each
    fdim = h_loc * hd           # local feature dim (512)
    FC = fdim // P              # proj contraction chunks (4)
    QT = 512                    # query tile
    QH = tok // QT              # query tiles per core (4)
    TC = QT // P                # token blocks per query tile (4)
    scale = float(hd) ** -0.5

    nc = bacc.Bacc("TRN2", target_bir_lowering=False, debug=debug)

    xT = nc.dram_tensor("xT", [d, tok], bf16, kind="ExternalInput")
    wq = nc.dram_tensor("wq", [d, fdim], bf16, kind="ExternalInput")
    wk = nc.dram_tensor("wk", [d, fdim], bf16, kind="ExternalInput")
    wv = nc.dram_tensor("wv", [d, fdim], bf16, kind="ExternalInput")
    wp = nc.dram_tensor("wp", [fdim, d], bf16, kind="ExternalInput")
    bias = nc.dram_tensor("bias", [P, d], f32, kind="ExternalInput")
    # bf16 partials: host upcasts + sums the TP pair in f32
    out = nc.dram_tensor("out", [tok, d], bf16, kind="ExternalOutput")

    with tile.TileContext(nc) as tc:
        with (
            tc.tile_pool(name="sing", bufs=1) as sing,
            tc.tile_pool(name="psS", bufs=2, space="PSUM") as psS,
            tc.tile_pool(name="psO", bufs=2, space="PSUM") as psO,
            tc.tile_pool(name="psW", bufs=2, space="PSUM") as psW,
            tc.tile_pool(name="atp", bufs=20) as atp,
            tc.tile_pool(name="work", bufs=4) as work,
            tc.tile_pool(name="outp", bufs=4) as outp,
            tc.tile_pool(name="dscr", bufs=8, space="DRAM") as dscr,
        ):
          # HAM warmup fodder: constant SBUF tile, written once, read by the
          # junk matmuls that open every repetition.
          junk = sing.tile([P, 512], bf16)
          nc.vector.memset(junk, 0.25)
          for _rep in range(repeat):
            # junk matmuls keep the PE busy (HAM stays at K=8/8) while the
            # per-rep input DMAs stream in; they have no DMA dependencies.
            for _w in range(8):
                wps = psW.tile([P, 512], f32, tag="ps", name="wps")
                nc.tensor.matmul(wps, junk[:, 0:128], junk,
                                 start=True, stop=True)

            # ---- resident loads, first-needed first ------------------------
            wk_sb = sing.tile([P, DC, fdim], bf16)
            wq_sb = sing.tile([P, DC, fdim], bf16)
            wv_sb = sing.tile([P, DC, fdim], bf16)
            xT_sb = sing.tile([P, DC, tok], bf16)
            dma_engs = (nc.sync, nc.gpsimd, nc.scalar)

            def wload(w_dram, w_sb, pr, eng):
                eng.dma_start(
                    out=w_sb[:, :, pr * P:(pr + 1) * P],
                    in_=w_dram[:, pr * P:(pr + 1) * P].rearrange(
                        "(c p) m -> p c m", p=P))

            def xload(ts, qn):
                t0 = ts * (tok // 4)
                t1 = t0 + tok // 4
                for c in range(DC):
                    eng = dma_engs[(qn + c) % 3]
                    eng.dma_start(
                        out=xT_sb[:, c, t0:t1],
                        in_=xT[c * P:(c + 1) * P, t0:t1])

            wload(wk, wk_sb, 0, nc.gpsimd)
            wload(wq, wq_sb, 0, nc.sync)
            nc.scalar.dma_start(
                out=wv_sb, in_=wv[:, :].rearrange("(c p) m -> p c m", p=P))
            xload(0, 0)
            xload(1, 1)
            for pr in range(1, NP):
                wload(wk, wk_sb, pr, nc.gpsimd)
                wload(wq, wq_sb, pr, nc.sync)
            xload(2, 2)
            xload(3, 0)
            wp_sb = sing.tile([P, FC, d], bf16)
            nc.sync.dma_start(out=wp_sb, in_=wp[:, :].rearrange("(c p) m -> p c m", p=P))
            bias_sb = sing.tile([P, d], f32)
            nc.gpsimd.dma_start(out=bias_sb, in_=bias[:, :])

            qT_sb = sing.tile([P, NP, tok], bf16)
            kT_sb = sing.tile([P, NP, tok], bf16)
            vaug_sb = sing.tile([P, KC, NP, 2, hd + 1], bf16)
            # only the ones column needs init; v copies fill [0:hd]
            nc.vector.memset(vaug_sb[:, :, :, :, hd], 1.0)
            attT_sb = sing.tile([P, 2, NP, QT], bf16)

            # warm the exp activation table while DMAs stream (no DMA dep)
            warm_in = work.tile([P, 1], f32, tag="warm_in")
            nc.vector.memset(warm_in, 0.0)
            warm = work.tile([P, 1], f32, tag="warm")
            nc.scalar.activation(warm, warm_in, Exp)

            def qk_pair(jobs):
                """jobs: list of (w_sb, dst, p, n0) 512-token chunks. Emits
                the dc contraction chains interleaved across two PSUM banks
                (back-to-back matmuls into one bank stall the PE). A single
                job is split into two half-token chunks instead."""
                if len(jobs) == 1:
                    w_sb, dst, p, n0 = jobs[0]
                    psA = psW.tile([P, 512], f32, tag="ps", name="psA")
                    psB = psW.tile([P, 512], f32, tag="ps", name="psB")
                    for c in range(DC):
                        for ps, o in ((psA, 0), (psB, 256)):
                            nc.tensor.matmul(
                                ps[:, 0:256],
                                w_sb[:, c, p * P:(p + 1) * P],
                                xT_sb[:, c, n0 + o:n0 + o + 256],
                                start=(c == 0),
                                stop=(c == DC - 1),
                            )
                    nc.vector.tensor_copy(dst[:, p, n0:n0 + 256], psA[:, 0:256])
                    nc.vector.tensor_copy(dst[:, p, n0 + 256:n0 + 512],
                                          psB[:, 0:256])
                    return
                assert len(jobs) == 2
                tiles = [psW.tile([P, 512], f32, tag="ps", name=f"psj{i}")
                         for i in range(2)]
                for c in range(DC):
                    for (w_sb, dst, p, n0), ps in zip(jobs, tiles):
                        nc.tensor.matmul(
                            ps,
                            w_sb[:, c, p * P:(p + 1) * P],
                            xT_sb[:, c, n0:n0 + 512],
                            start=(c == 0),
                            stop=(c == DC - 1),
                        )
                for (w_sb, dst, p, n0), ps in zip(jobs, tiles):
                    nc.vector.tensor_copy(dst[:, p, n0:n0 + 512], ps)

            def v_pair(tb_a, tb_b):
                """vaug[:, tb, :, :, 0:hd] for two token blocks, all pairs at
                once (N=512 streams), chains interleaved across two banks."""
                tiles = [psW.tile([P, 512], f32, tag="ps", name=f"psv{i}")
                         for i in range(2)]
                for c in range(DC):
                    for tb, ps in zip((tb_a, tb_b), tiles):
                        nc.tensor.matmul(
                            ps,
                            xT_sb[:, c, tb * P:(tb + 1) * P],
                            wv_sb[:, c, :],
                            start=(c == 0),
                            stop=(c == DC - 1),
                        )
                for tb, ps in zip((tb_a, tb_b), tiles):
                    nc.vector.tensor_copy(
                        vaug_sb[:, tb, :, :, 0:hd],
                        ps.rearrange("p (pr e f) -> p pr e f", pr=NP, e=2),
                    )

            def proj_tb(qh_, tb):
                """out[tb block] = attT.T @ wp + bias. Both 512-wide halves
                are emitted adjacently per stationary chunk (weight reuse),
                chains interleaved across two PSUM banks."""
                r0 = qh_ * QT + tb * P
                pps = [psW.tile([P, 512], f32, tag="ps", name=f"pp{i}")
                       for i in range(2)]
                for fc in range(FC):
                    for i, pp in enumerate(pps):
                        nc.tensor.matmul(
                            pp,
                            attT_sb[:, qh_ % 2, fc, tb * P:(tb + 1) * P],
                            wp_sb[:, fc, i * 512:(i + 1) * 512],
                            start=(fc == 0),
                            stop=(fc == FC - 1),
                        )
                for i, pp in enumerate(pps):
                    ot = outp.tile([P, 512], bf16, tag="ot")
                    nc.vector.tensor_add(ot, pp, bias_sb[:, i * 512:(i + 1) * 512])
                    oeng = nc.sync if (tb + i) % 2 == 0 else nc.gpsimd
                    oeng.dma_start(out=out[r0:r0 + P, i * 512:(i + 1) * 512],
                                   in_=ot)

            def s_mega_pair(p, q0, j):
                ss = [psS.tile([P, 2, QT], f32, tag="ss", name=f"ss{e}")
                      for e in range(2)]
                for jj in range(2):
                    kc = 2 * j + jj
                    # explicit tile_position + per-instruction head
                    # interleave: the e0/e1 matmuls land on different PE
                    # row-groups and overlap
                    for e in range(2):
                        nc.tensor.matmul(
                            ss[e][:, jj, :],
                            kT_sb[e * hd:(e + 1) * hd, p, kc * P:(kc + 1) * P],
                            qT_sb[e * hd:(e + 1) * hd, p, q0:q0 + QT],
                            start=True,
                            stop=True,
                            tile_position=(e * hd, 0),
                        )
                ats = []
                for e in range(2):
                    at = atp.tile([P, 2, QT], bf16, tag="at", name=f"at{e}")
                    nc.scalar.activation(at, ss[e], Exp, scale=scale)
                    ats.append(at)
                return tuple(ats)

            def av_chunk(po, p, at_list, kc0, n):
                """n steps of both heads' A@V accumulation chains, emitted
                head-interleaved so consecutive matmuls target different
                PSUM banks (same-bank back-to-back stalls the PE)."""
                for kc in range(kc0, kc0 + n):
                    for e in range(2):
                        nc.tensor.matmul(
                            po[e][0:hd + 1, :],
                            vaug_sb[:, kc, p, e, :],
                            at_list[kc // 2][e][:, kc % 2, :],
                            start=(kc == 0),
                            stop=(kc == KC - 1),
                        )

            def tail(pend):
                # normalize: den rows -> DRAM -> stride-0 partition-broadcast
                # read -> ONE [128, 512] reciprocal -> in-place multiply.
                # The unnormalized po is copied into attT up front so the
                # PSUM banks free without waiting on the DMA round trip.
                qh_t, p_t, po = pend[0], pend[1], pend[2]
                sums_dr = dscr.tile([2, QT], f32, tag="sums_dr")
                att = [attT_sb[e * hd:(e + 1) * hd, qh_t % 2, p_t, :]
                       for e in range(2)]
                for e in range(2):
                    den_sb = work.tile([hd + 1, QT], f32, tag=f"den{e}")
                    nc.vector.tensor_copy(
                        den_sb[hd:hd + 1, :], po[e][hd:hd + 1, :])
                    nc.vector.tensor_copy(att[e], po[e][0:hd, :])
                    nc.gpsimd.dma_start(out=sums_dr[e:e + 1, :],
                                        in_=den_sb[hd:hd + 1, :])
                rec_src = work.tile([P, QT], f32, tag="rsrc")
                for e in range(2):
                    nc.gpsimd.dma_start(
                        out=rec_src[e * hd:(e + 1) * hd, :],
                        in_=sums_dr[e:e + 1, :].partition_broadcast(hd))
                rec = work.tile([P, QT], f32, tag="rec")
                nc.vector.reciprocal_approx_fast(rec, rec_src)
                for e in range(2):
                    nc.vector.tensor_mul(
                        att[e], att[e], rec[e * hd:(e + 1) * hd, :])

            # ---- main loop: software-pipelined combos ----------------------
            # Per combo, S megas stream so ScalarE never starves; the
            # previous combo's A@V rides along 6 matmuls per mega, its tail
            # lands at mega 5, and the next combo's q/k chunks plus the
            # (remapped) proj blocks fill the remaining PE slots.
            combos = [(qh, p) for qh in range(QH) for p in range(NP)]
            qk_pair([(wk_sb, kT_sb, 0, 0), (wq_sb, qT_sb, 0, 0)])
            for g in range(4):
                v_pair(2 * g, 2 * g + 1)
            prev = None   # (qh, p, at_list): combo whose A@V+tail is deferred
            for ci, (qh, p) in enumerate(combos):
                q0 = qh * QT
                at_list = []
                po_prev = None
                for j in range(NMEG):
                    at_list.append(s_mega_pair(p, q0, j))
                    if prev is not None:
                        if j == 0:
                            po_prev = [psO.tile([P, QT], f32, tag="po",
                                                name=f"po{e}")
                                       for e in range(2)]
                        if j <= 4:
                            av_chunk(po_prev, prev[1], prev[2], 3 * j, 3)
                        elif j == 5:
                            av_chunk(po_prev, prev[1], prev[2], 15, 1)
                            tail((prev[0], prev[1], po_prev))
                    if j == 1 and qh == 0:
                        qk_pair([(wk_sb, kT_sb, p, 512)])
                    elif j == 2:
                        if qh == 0:
                            qk_pair([(wk_sb, kT_sb, p, 1024)])
                        if ci + 1 < len(combos):
                            qh_n, p_n = combos[ci + 1]
                            qk_pair([(wq_sb, qT_sb, p_n, qh_n * QT)])
                    elif j == 3:
                        if qh == 0:
                            jobs = [(wk_sb, kT_sb, p, 1536)]
                            if p + 1 < NP:
                                jobs.append((wk_sb, kT_sb, p + 1, 0))
                            qk_pair(jobs)
                        elif p >= 1:
                            proj_tb(qh - 1, p - 1)
                        elif qh >= 2:
                            proj_tb(qh - 2, 3)
                    elif j == 5 and qh == 0 and ci == 0:
                        v_pair(8, 9)
                        v_pair(10, 11)
                    elif j == 6 and qh == 0 and ci == 0:
                        v_pair(12, 13)
                        v_pair(14, 15)
                prev = (qh, p, at_list)
            # epilogue: final combo's A@V interleaved with the independent
            # proj block, tail, then the last query tile's proj
            po_prev = [psO.tile([P, QT], f32, tag="po", name=f"po{e}")
                       for e in range(2)]
            av_chunk(po_prev, prev[1], prev[2], 0, 8)
            proj_tb(QH - 2, 3)
            av_chunk(po_prev, prev[1], prev[2], 8, 8)
            tail((prev[0], prev[1], po_prev))
            for tb in range(TC):
                proj_tb(QH - 1, tb)

    nc.compile()
    return nc


def get_program(tok=N, d=D, h_loc=HLOC, hd=HD, debug=False, repeat=1):
    key = (tok, d, h_loc, hd, debug, repeat)
    if key not in _PROG_CACHE:
        _PROG_CACHE[key] = _build_program(tok, d, h_loc, hd, debug=debug,
                                          repeat=repeat)
    return _PROG_CACHE[key]


def make_in_maps(inputs_f32, W_qkv, W_proj, b_proj):
    """Shard full inputs into the 8 per-core input dicts."""
    import ml_dtypes

    bf16 = ml_dtypes.bfloat16
    in_maps = []
    for core in range(NCORES):
        b_idx, j = divmod(core, TP)
        f0, f1 = j * FDIM, (j + 1) * FDIM
        xT = np.ascontiguousarray(inputs_f32[b_idx].T).astype(bf16)
        wq_s = np.ascontiguousarray(W_qkv[:, f0:f1]).astype(bf16)
        wk_s = np.ascontiguousarray(W_qkv[:, D + f0:D + f1]).astype(bf16)
        wv_s = np.ascontiguousarray(W_qkv[:, 2 * D + f0:2 * D + f1]).astype(bf16)
        wp_s = np.ascontiguousarray(W_proj[f0:f1, :]).astype(bf16)
        if j == 0:
            bias_rep = np.broadcast_to(b_proj.astype(np.float32), (128, D)).copy()
        else:
            bias_rep = np.zeros((128, D), np.float32)
        in_maps.append(
            {"xT": xT, "wq": wq_s, "wk": wk_s, "wv": wv_s, "wp": wp_s,
             "bias": bias_rep}
        )
    return in_maps


def kernel(inputs, W_qkv, W_proj, b_proj):
    from concourse.bass_utils import run_bass_kernel_spmd

    inputs = np.asarray(inputs, dtype=np.float32)
    W_qkv = np.asarray(W_qkv, dtype=np.float32)
    W_proj = np.asarray(W_proj, dtype=np.float32)
    b_proj = np.asarray(b_proj, dtype=np.float32)

    nc = get_program()
    in_maps = make_in_maps(inputs, W_qkv, W_proj, b_proj)
    res = run_bass_kernel_spmd(nc, in_maps, core_ids=list(range(NCORES)))
    outs = [r["out"].astype(np.float32) for r in res.results]
    full = np.stack([outs[TP * b] + outs[TP * b + 1] for b in range(B)], axis=0)
    return full
